# revision 28
# baseline (speedup 1.0000x reference)
"""PerceiverAttention kernel for Trainium2 (8 NeuronCores, data-parallel over batch).

Per core: one batch element. Flash-style streaming over the 8192 media rows:
LN-stats (bn_stats) -> LN-apply -> PE-transpose -> K^T/V projections (float32r)
-> per-head-pair sim matmul (block-diag q stationary) -> additive mask -> exp
(ACT, fused denominator accumulation) -> PE-transpose of attn -> accumulating
out matmuls into a persistent PSUM bank. Epilogue normalizes by the softmax
denominator and applies Wout.

gamma/beta/kv_gate/softmax-scale are folded on the host into the weight
tensors, bias vectors and an additive mask, so the device kernel only sees
plain matmuls plus a vanilla LayerNorm (gamma=1, beta=0 equivalent).
"""

import sys

if "/opt/trn_rl_repo" not in sys.path:
    sys.path.insert(0, "/opt/trn_rl_repo")

import numpy as np

B = 8
F = 8192
NQ = 64          # latents (queries)
D = 1024         # model dim
INNER = 512      # heads * dim_head
HEADS = 8
DH = 64
SCALE = DH ** -0.5
EPS = 1e-5
NEG = -1e30
NCORES = 8

R_TILE = 512             # kv rows per streaming tile
NT = F // R_TILE         # 16 tiles of media rows
RC = R_TILE // 128       # 4 row-chunks per tile
CC = D // 128            # 8 contraction chunks (model dim)
JJ = INNER // 128        # 4 inner chunks == 4 head pairs
MM = D // 128            # 8 output chunks

_cache = {}


def _build(gate, same_gamma, has_bm, has_bl, dbg=False):
    import concourse.bass as bass
    import concourse.tile as tile
    import concourse.mybir as mybir
    from concourse import bacc
    from contextlib import ExitStack

    f32 = mybir.dt.float32
    f32r = mybir.dt.float32r
    AF = mybir.ActivationFunctionType
    OP = mybir.AluOpType
    AX = mybir.AxisListType

    nc = bacc.Bacc()

    x_d = nc.dram_tensor("x", [F, D], f32, kind="ExternalInput")
    lat_d = nc.dram_tensor("lat", [NQ, D], f32, kind="ExternalInput")
    mask_d = nc.dram_tensor("maskoff", [F], f32, kind="ExternalInput")
    wk_d = nc.dram_tensor("wk", [D, INNER], f32r, kind="ExternalInput")
    wv_d = nc.dram_tensor("wv", [D, INNER], f32r, kind="ExternalInput")
    wq_d = nc.dram_tensor("wq", [D, INNER], f32r, kind="ExternalInput")
    wout_d = nc.dram_tensor("wout", [INNER, D], f32r, kind="ExternalInput")
    ident_d = nc.dram_tensor("ident", [128, 128], f32, kind="ExternalInput")
    if not same_gamma:
        wkl_d = nc.dram_tensor("wkl", [D, INNER], f32r, kind="ExternalInput")
        wvl_d = nc.dram_tensor("wvl", [D, INNER], f32r, kind="ExternalInput")
    if has_bm:
        bkm_d = nc.dram_tensor("bkm", [INNER], f32, kind="ExternalInput")
        bvm_d = nc.dram_tensor("bvm", [INNER], f32, kind="ExternalInput")
    if has_bl:
        bkl_d = nc.dram_tensor("bkl", [INNER], f32, kind="ExternalInput")
        bvl_d = nc.dram_tensor("bvl", [INNER], f32, kind="ExternalInput")
        bq_d = nc.dram_tensor("bq", [INNER], f32, kind="ExternalInput")
    out_d = nc.dram_tensor("out", [NQ, D], f32, kind="ExternalOutput")
    if dbg:
        dbg_sums_d = nc.dram_tensor("dbg_sums", [128, JJ, NT + 1], f32, kind="ExternalOutput")
        dbg_qT_d = nc.dram_tensor("dbg_qT", [128, JJ, NQ], f32, kind="ExternalOutput")
        dbg_ktlat_d = nc.dram_tensor("dbg_ktlat", [128, JJ, NQ], f32, kind="ExternalOutput")
        dbg_vlat_d = nc.dram_tensor("dbg_vlat", [64, INNER], f32, kind="ExternalOutput")
        dbg_oT_d = nc.dram_tensor("dbg_oT", [128, JJ, 64], f32, kind="ExternalOutput")
        dbg_kt0_d = nc.dram_tensor("dbg_kt0", [128, JJ, R_TILE], f32, kind="ExternalOutput")
        dbg_vt0_d = nc.dram_tensor("dbg_vt0", [128, RC, INNER], f32, kind="ExternalOutput")
        dbg_po_d = nc.dram_tensor("dbg_po", [128, 512], f32, kind="ExternalOutput")
        dbg_at0_d = nc.dram_tensor("dbg_at0", [128, JJ, R_TILE], f32, kind="ExternalOutput")
        dbg_att0_d = nc.dram_tensor("dbg_att0", [128, JJ, RC, 128], f32, kind="ExternalOutput")

    with tile.TileContext(nc) as tc, ExitStack() as ctx:
        singles = ctx.enter_context(tc.tile_pool(name="singles", bufs=1))
        xpool = ctx.enter_context(tc.tile_pool(name="xpool", bufs=4))
        xtpool = ctx.enter_context(tc.tile_pool(name="xtpool", bufs=1))
        ktpool = ctx.enter_context(tc.tile_pool(name="ktpool", bufs=2))
        vpool = ctx.enter_context(tc.tile_pool(name="vpool", bufs=2))
        apool = ctx.enter_context(tc.tile_pool(name="apool", bufs=2))
        atpool = ctx.enter_context(tc.tile_pool(name="atpool", bufs=2))
        mpool = ctx.enter_context(tc.tile_pool(name="mpool", bufs=2))
        spool = ctx.enter_context(tc.tile_pool(name="spool", bufs=4))
        # PSUM: pT (transposes + sim + attn-T), pK (K^T/q/Wout), pV (V/final-T), pO (out accum)
        pT = ctx.enter_context(tc.tile_pool(name="pT", bufs=2, space="PSUM"))
        pK = ctx.enter_context(tc.tile_pool(name="pK", bufs=2, space="PSUM"))
        pV = ctx.enter_context(tc.tile_pool(name="pV", bufs=2, space="PSUM"))
        pO = ctx.enter_context(tc.tile_pool(name="pO", bufs=1, space="PSUM"))

        # ---------- constants / weights (ident DMA'd LAST; a dummy PE
        # transpose then absorbs the whole prologue DMA queue into PE's
        # vector clock so matmuls only ever need their one cross-engine wait).
        ones1 = singles.tile([1, 128], f32, tag="ones1")
        nc.vector.memset(ones1, 1.0)
        epsc = singles.tile([128, 1], f32, tag="epsc")
        nc.vector.memset(epsc, EPS)
        zeroc = singles.tile([128, 1], f32, tag="zeroc")
        nc.vector.memset(zeroc, 0.0)

        wk_sb = singles.tile([128, CC, INNER], f32r, tag="wk")
        nc.sync.dma_start(out=wk_sb, in_=wk_d[:, :].rearrange("(c p) j -> p c j", p=128))
        wv_sb = singles.tile([128, CC, INNER], f32r, tag="wv")
        nc.sync.dma_start(out=wv_sb, in_=wv_d[:, :].rearrange("(c p) j -> p c j", p=128))
        wq_sb = singles.tile([128, CC, INNER], f32r, tag="wq")
        nc.sync.dma_start(out=wq_sb, in_=wq_d[:, :].rearrange("(c p) j -> p c j", p=128))
        wout_sb = singles.tile([128, JJ, D], f32r, tag="wout")
        nc.sync.dma_start(out=wout_sb, in_=wout_d[:, :].rearrange("(c p) m -> p c m", p=128))
        if not same_gamma:
            wkl_sb = singles.tile([128, CC, INNER], f32r, tag="wkl")
            nc.sync.dma_start(out=wkl_sb, in_=wkl_d[:, :].rearrange("(c p) j -> p c j", p=128))
            wvl_sb = singles.tile([128, CC, INNER], f32r, tag="wvl")
            nc.sync.dma_start(out=wvl_sb, in_=wvl_d[:, :].rearrange("(c p) j -> p c j", p=128))
        else:
            wkl_sb, wvl_sb = wk_sb, wv_sb
        if has_bm:
            bkm_sb = singles.tile([128, JJ], f32, tag="bkm")
            nc.sync.dma_start(out=bkm_sb, in_=bkm_d[:].rearrange("(c p) -> p c", p=128))
            bvm_row = singles.tile([1, INNER], f32, tag="bvm")
            nc.sync.dma_start(out=bvm_row, in_=bvm_d[:].rearrange("j -> 1 j"))
        if has_bl:
            bkl_sb = singles.tile([128, JJ], f32, tag="bkl")
            nc.sync.dma_start(out=bkl_sb, in_=bkl_d[:].rearrange("(c p) -> p c", p=128))
            bvl_row = singles.tile([1, INNER], f32, tag="bvl")
            nc.sync.dma_start(out=bvl_row, in_=bvl_d[:].rearrange("j -> 1 j"))
            bq_sb = singles.tile([128, JJ], f32, tag="bq")
            nc.sync.dma_start(out=bq_sb, in_=bq_d[:].rearrange("(c p) -> p c", p=128))
        lat_t = singles.tile([64, D], f32, tag="lat_t")
        nc.sync.dma_start(out=lat_t, in_=lat_d[:, :])
        ident = singles.tile([128, 128], f32, tag="ident")
        nc.sync.dma_start(out=ident, in_=ident_d[:, :])
        # dummy transpose: pulls PE's DMA-queue clock up to the last prologue DMA
        dummy_ps = pT.tile([128, 512], f32, tag="pT")
        nc.tensor.transpose(dummy_ps[:, 0:128], ident, ident)

        sums = singles.tile([128, JJ, NT + 1], f32, tag="sums")
        qTp = singles.tile([128, JJ, 128], f32r, tag="qTp")
        latT = singles.tile([128, CC, NQ], f32r, tag="latT")
        latTk = singles.tile([128, CC, NQ], f32r, tag="latTk")
        qT = singles.tile([128, JJ, NQ], f32r, tag="qT")
        kT_lat = singles.tile([128, JJ, NQ], f32r, tag="kTlat")
        v_lat = singles.tile([64, INNER], f32, tag="vlat")
        oT = singles.tile([128, JJ, 64], f32r, tag="oT")
        fT = singles.tile([128, MM, 64], f32, tag="fT")
        outf = singles.tile([64, D], f32, tag="outf")
        rec = singles.tile([128, JJ], f32, tag="rec")
        rrow = singles.tile([1, JJ, 128], f32, tag="rrow")
        Rsb = singles.tile([128, JJ, 128], f32, tag="Rsb")

        pOt = pO.tile([128, 512], f32, tag="pO")

        def ln_stats(src, parts, tagpfx):
            st = spool.tile([parts, 2, 6], f32, tag=tagpfx + "st")
            nc.vector.bn_stats(out=st[:, 0, :], in_=src[:, 0:512])
            nc.vector.bn_stats(out=st[:, 1, :], in_=src[:, 512:1024])
            mv = spool.tile([parts, 2], f32, tag=tagpfx + "mv")
            nc.vector.bn_aggr(out=mv, in_=st)
            sd = spool.tile([parts, 1], f32, tag=tagpfx + "sd")
            # sqrt(var+eps) = exp(0.5*ln(var+eps)); Ln+Exp share one ACT table
            # set (no table thrash vs the softmax Exp). Final value comes from
            # DVE reciprocal so the LN-apply tensor_scalar has DVE-only
            # producers (TensorScalar ISA struct has a single wait slot).
            nc.scalar.activation(out=sd, in_=mv[:, 1:2], func=AF.Ln,
                                 bias=epsc[:parts, :], scale=1.0)
            nc.scalar.activation(out=sd, in_=sd, func=AF.Exp,
                                 bias=zeroc[:parts, :], scale=0.5)
            rs = spool.tile([parts, 1], f32, tag=tagpfx + "rs")
            nc.vector.reciprocal(out=rs, in_=sd)
            return mv, rs

        # ---------- latents prologue ----------
        lmv, lrs = ln_stats(lat_t, 64, "l")
        lat_s = singles.tile([64, D], f32, tag="lat_s")
        nc.vector.tensor_scalar(
            out=lat_s, in0=lat_t, scalar1=lmv[:, 0:1], scalar2=lrs,
            op0=OP.subtract, op1=OP.mult)

        def transpose_1024_to_T(src, dst):
            # src [64, 1024] -> dst [128, CC, 64]
            for g in range(2):
                ptile = pT.tile([128, 512], f32, tag="pT")
                for c in range(4):
                    cc = g * 4 + c
                    nc.tensor.transpose(
                        ptile[:, c * 64:(c + 1) * 64],
                        src[:, cc * 128:(cc + 1) * 128],
                        ident[0:64, 0:64])
                nc.scalar.copy(
                    out=dst[:, g * 4:(g + 1) * 4, :],
                    in_=ptile[:, 0:256].rearrange("p (c q) -> p c q", q=64))

        transpose_1024_to_T(lat_s, latT)
        if same_gamma:
            # kv latent rows are gate * LN(lat): fold gate into the LN scale
            lrsg = spool.tile([64, 1], f32, tag="lrsg")
            nc.vector.tensor_scalar_mul(out=lrsg, in0=lrs, scalar1=float(gate))
            latkv_s = singles.tile([64, D], f32, tag="latkv_s")
            nc.vector.tensor_scalar(
                out=latkv_s, in0=lat_t, scalar1=lmv[:, 0:1], scalar2=lrsg,
                op0=OP.subtract, op1=OP.mult)
            transpose_1024_to_T(latkv_s, latTk)
            latkv = latTk
        else:
            latkv = latT  # gate folded into wkl/wvl on the host

        # q^T = (wq^T @ latT) * SCALE (+ bias)
        for jjc in range(JJ):
            pq = pK.tile([128, 512], f32, tag="pK")
            for c in range(CC):
                nc.tensor.matmul(
                    pq[:, 0:NQ],
                    lhsT=wq_sb[:, c, jjc * 128:(jjc + 1) * 128],
                    rhs=latT[:, c, :],
                    start=(c == 0), stop=(c == CC - 1))
            if has_bl:
                nc.scalar.activation(
                    out=qT[:, jjc, :], in_=pq[:, 0:NQ], func=AF.Identity,
                    bias=bq_sb[:, jjc:jjc + 1], scale=SCALE)
            else:
                nc.scalar.mul(out=qT[:, jjc, :], in_=pq[:, 0:NQ], mul=SCALE)

        # block-diagonal stationary per head pair
        nc.vector.memset(qTp.bitcast(f32), 0.0)
        for p in range(JJ):
            nc.vector.tensor_copy(out=qTp[0:64, p, 0:64], in_=qT[0:64, p, :])
            nc.vector.tensor_copy(out=qTp[64:128, p, 64:128], in_=qT[64:128, p, :])

        # ---------- streaming over media rows ----------
        for t in range(NT):
            xsT = xtpool.tile([128, CC, R_TILE], f32r, tag="xsT")
            ktt = ktpool.tile([128, JJ, R_TILE], f32r, tag="ktt")
            vt = vpool.tile([128, RC, INNER], f32, tag="vt")
            mt = mpool.tile([128, R_TILE], f32, tag="mt")
            msl = mask_d[t * R_TILE:(t + 1) * R_TILE]
            nc.gpsimd.dma_start(
                out=mt,
                in_=bass.AP(tensor=msl.tensor, offset=msl.offset,
                            ap=[[0, 128]] + list(msl.ap)))

            for r in range(RC):
                xt = xpool.tile([128, D], f32, tag="xt")
                row0 = t * R_TILE + r * 128
                nc.sync.dma_start(out=xt, in_=x_d[row0:row0 + 128, :])
                mv, rs = ln_stats(xt, 128, "x")
                xs = xpool.tile([128, D], f32, tag="xs")
                nc.vector.tensor_scalar(
                    out=xs, in0=xt, scalar1=mv[:, 0:1], scalar2=rs,
                    op0=OP.subtract, op1=OP.mult)
                for g in range(2):
                    ptile = pT.tile([128, 512], f32, tag="pT")
                    for c in range(4):
                        cc = g * 4 + c
                        nc.tensor.transpose(
                            ptile[:, c * 128:(c + 1) * 128],
                            xs[:, cc * 128:(cc + 1) * 128],
                            ident)
                    nc.vector.tensor_copy(
                        out=xsT[:, g * 4:(g + 1) * 4, r * 128:(r + 1) * 128],
                        in_=ptile.rearrange("p (c r) -> p c r", r=128))

            # K^T tile: [128(j), JJ, R_TILE]
            for jjc in range(JJ):
                pk = pK.tile([128, 512], f32, tag="pK")
                for c in range(CC):
                    nc.tensor.matmul(
                        pk,
                        lhsT=wk_sb[:, c, jjc * 128:(jjc + 1) * 128],
                        rhs=xsT[:, c, :],
                        start=(c == 0), stop=(c == CC - 1))
                if has_bm:
                    nc.scalar.activation(
                        out=ktt[:, jjc, :], in_=pk, func=AF.Identity,
                        bias=bkm_sb[:, jjc:jjc + 1], scale=1.0)
                else:
                    nc.scalar.copy(out=ktt[:, jjc, :], in_=pk)

            # V tile: [128(r), RC, INNER]
            for r in range(RC):
                pv = pV.tile([128, 512], f32, tag="pV")
                first = True
                if has_bm:
                    nc.tensor.matmul(pv, lhsT=ones1, rhs=bvm_row,
                                     start=True, stop=False)
                    first = False
                for c in range(CC):
                    nc.tensor.matmul(
                        pv,
                        lhsT=xsT[:, c, r * 128:(r + 1) * 128],
                        rhs=wv_sb[:, c, :],
                        start=first, stop=(c == CC - 1))
                    first = False
                nc.scalar.copy(out=vt[:, r, :], in_=pv)

            # attention per head pair
            for p in range(JJ):
                ps = pT.tile([128, 512], f32, tag="pT")
                nc.tensor.matmul(
                    ps, lhsT=qTp[:, p, :],
                    rhs=ktt[:, p, :], start=True, stop=True)
                nc.vector.tensor_tensor(out=ps, in0=ps, in1=mt, op=OP.add)
                at = apool.tile([128, R_TILE], f32, tag="at")
                nc.scalar.activation(out=at, in_=ps, func=AF.Exp, bias=zeroc,
                                     accum_out=sums[:, p, t:t + 1])
                att = atpool.tile([128, RC, 128], f32, tag="att")
                pt2 = pT.tile([128, 512], f32, tag="pT")
                for s in range(RC):
                    nc.tensor.transpose(
                        pt2[:, s * 128:(s + 1) * 128],
                        at[:, s * 128:(s + 1) * 128], ident)
                nc.scalar.copy(
                    out=att, in_=pt2.rearrange("p (s q) -> p s q", q=128))
                if dbg and t == 0:
                    nc.sync.dma_start(out=dbg_at0_d[:, p, :], in_=at)
                    nc.sync.dma_start(out=dbg_att0_d[:, p, :, :], in_=att)
                for s in range(RC):
                    # single start for the whole bank: start=True arms a
                    # pending-zero over the full 2KB zero-region, so only the
                    # very first accumulating matmul may carry it.
                    nc.tensor.matmul(
                        pOt[:, p * 128:(p + 1) * 128],
                        lhsT=vt[:, s, p * 128:(p + 1) * 128],
                        rhs=att[:, s, :],
                        start=(t == 0 and s == 0 and p == 0), stop=False,
                        skip_group_check=True)
            if dbg and t == 0:
                nc.sync.dma_start(out=dbg_kt0_d[:, :, :], in_=ktt.bitcast(f32))
                nc.sync.dma_start(out=dbg_vt0_d[:, :, :], in_=vt)

        # ---------- latent kv rows (always valid, gated) ----------
        for jjc in range(JJ):
            pk = pK.tile([128, 512], f32, tag="pK")
            for c in range(CC):
                nc.tensor.matmul(
                    pk[:, 0:NQ],
                    lhsT=wkl_sb[:, c, jjc * 128:(jjc + 1) * 128],
                    rhs=latkv[:, c, :],
                    start=(c == 0), stop=(c == CC - 1))
            if has_bl:
                nc.scalar.activation(
                    out=kT_lat[:, jjc, :], in_=pk[:, 0:NQ], func=AF.Identity,
                    bias=bkl_sb[:, jjc:jjc + 1], scale=1.0)
            else:
                nc.scalar.copy(out=kT_lat[:, jjc, :], in_=pk[:, 0:NQ])

        pv = pV.tile([128, 512], f32, tag="pV")
        first = True
        if has_bl:
            nc.tensor.matmul(pv[0:64, :], lhsT=ones1[:, 0:64], rhs=bvl_row,
                             start=True, stop=False)
            first = False
        for c in range(CC):
            nc.tensor.matmul(
                pv[0:64, :],
                lhsT=latkv[:, c, :],
                rhs=wvl_sb[:, c, :],
                start=first, stop=(c == CC - 1))
            first = False
        nc.scalar.copy(out=v_lat, in_=pv[0:64, :])

        for p in range(JJ):
            ps = pT.tile([128, 512], f32, tag="pT")
            nc.tensor.matmul(
                ps[:, 0:NQ], lhsT=qTp[:, p, :],
                rhs=kT_lat[:, p, :], start=True, stop=True)
            at = apool.tile([128, R_TILE], f32, tag="at")
            nc.scalar.activation(out=at[:, 0:NQ], in_=ps[:, 0:NQ], func=AF.Exp,
                                 bias=zeroc, accum_out=sums[:, p, NT:NT + 1])
            att = atpool.tile([128, RC, 128], f32, tag="att")
            pt2 = pT.tile([128, 512], f32, tag="pT")
            nc.tensor.transpose(pt2[0:64, 0:128], at[:, 0:NQ], ident)
            nc.scalar.copy(out=att[0:64, 0, :], in_=pt2[0:64, 0:128])
            nc.tensor.matmul(
                pOt[:, p * 128:(p + 1) * 128],
                lhsT=v_lat[:, p * 128:(p + 1) * 128],
                rhs=att[0:64, 0, :],
                start=False, stop=True, skip_group_check=True)

        # ---------- epilogue ----------
        for p in range(JJ):
            nc.vector.reduce_sum(out=rec[:, p:p + 1], in_=sums[:, p, :], axis=AX.X)
        for p in range(JJ):
            nc.vector.reciprocal(out=rec[:, p:p + 1], in_=rec[:, p:p + 1])
        prs = []
        for p in range(JJ):
            pr = pT.tile([128, 512], f32, tag="pT")
            nc.tensor.transpose(pr[0:1, 0:128], rec[:, p:p + 1], ident)
            nc.scalar.copy(out=rrow[:, p, :], in_=pr[0:1, 0:128])
        for p in range(JJ):
            pR = pK.tile([128, 512], f32, tag="pK")
            nc.tensor.matmul(pR[:, 0:128], lhsT=ones1, rhs=rrow[:, p, :],
                             start=True, stop=True)
            nc.vector.tensor_copy(out=Rsb[:, p, :], in_=pR[:, 0:128])
        if dbg:
            dbg_po_sb = singles.tile([128, 512], f32, tag="dbgpo")
            nc.scalar.copy(out=dbg_po_sb, in_=pOt)
            nc.sync.dma_start(out=dbg_po_d[:, :], in_=dbg_po_sb)
        for p in range(JJ):
            nc.vector.tensor_tensor(
                out=oT[0:64, p, :], in0=pOt[0:64, p * 128:p * 128 + 64],
                in1=Rsb[0:64, p, 0:64], op=OP.mult)
            nc.vector.tensor_tensor(
                out=oT[64:128, p, :], in0=pOt[64:128, p * 128 + 64:p * 128 + 128],
                in1=Rsb[64:128, p, 64:128], op=OP.mult)

        for m in range(MM):
            pf = pK.tile([128, 512], f32, tag="pK")
            for c in range(JJ):
                nc.tensor.matmul(
                    pf[:, 0:64],
                    lhsT=wout_sb[:, c, m * 128:(m + 1) * 128],
                    rhs=oT[:, c, :],
                    start=(c == 0), stop=(c == JJ - 1))
            nc.scalar.copy(out=fT[:, m, :], in_=pf[:, 0:64])

        for g in range(2):
            pfin = pV.tile([128, 512], f32, tag="pV")
            for s in range(4):
                m = g * 4 + s
                nc.tensor.transpose(pfin[0:64, s * 128:(s + 1) * 128],
                                    fT[:, m, :], ident)
            nc.scalar.copy(out=outf[:, g * 512:(g + 1) * 512], in_=pfin[0:64, :])
        nc.sync.dma_start(out=out_d[:, :], in_=outf)
        if dbg:
            nc.sync.dma_start(out=dbg_sums_d[:, :, :], in_=sums)
            nc.sync.dma_start(out=dbg_qT_d[:, :, :], in_=qT.bitcast(f32))
            nc.sync.dma_start(out=dbg_ktlat_d[:, :, :], in_=kT_lat.bitcast(f32))
            nc.sync.dma_start(out=dbg_vlat_d[:, :], in_=v_lat)
            nc.sync.dma_start(out=dbg_oT_d[:, :, :], in_=oT.bitcast(f32))

    nc.finalize()
    return nc


def _prep(inputs):
    x = np.ascontiguousarray(np.asarray(inputs["x"], dtype=np.float32))
    lat = np.ascontiguousarray(np.asarray(inputs["latents"], dtype=np.float32))
    mask = np.asarray(inputs["key_padding_mask"])
    gate = float(np.asarray(inputs["kv_gate"]))
    gm = np.asarray(inputs["gamma_media"], dtype=np.float32)
    bm = np.asarray(inputs["beta_media"], dtype=np.float32)
    gl = np.asarray(inputs["gamma_lat"], dtype=np.float32)
    bl = np.asarray(inputs["beta_lat"], dtype=np.float32)
    Wq = np.asarray(inputs["Wq"], dtype=np.float32)
    Wk = np.asarray(inputs["Wk"], dtype=np.float32)
    Wv = np.asarray(inputs["Wv"], dtype=np.float32)
    Wout = np.ascontiguousarray(np.asarray(inputs["Wout"], dtype=np.float32))

    same_gamma = bool(np.array_equal(gm, gl))
    has_bm = bool(np.any(bm))
    has_bl = bool(np.any(bl))

    gWk = np.ascontiguousarray(gm[:, None] * Wk)
    gWv = np.ascontiguousarray(gm[:, None] * Wv)
    gWq = np.ascontiguousarray(gl[:, None] * Wq)
    maskoff = np.where(mask, np.float32(NEG), np.float32(0.0)).astype(np.float32)
    maskoff = np.ascontiguousarray(maskoff)

    per_core_common = {
        "wk": gWk, "wv": gWv, "wq": gWq, "wout": Wout,
    }
    if not same_gamma:
        per_core_common["wkl"] = np.ascontiguousarray(gate * gl[:, None] * Wk)
        per_core_common["wvl"] = np.ascontiguousarray(gate * gl[:, None] * Wv)
    if has_bm:
        per_core_common["bkm"] = np.ascontiguousarray(bm @ Wk)
        per_core_common["bvm"] = np.ascontiguousarray(bm @ Wv)
    if has_bl:
        per_core_common["bkl"] = np.ascontiguousarray(gate * (bl @ Wk))
        per_core_common["bvl"] = np.ascontiguousarray(gate * (bl @ Wv))
        per_core_common["bq"] = np.ascontiguousarray(bl @ Wq)

    per_core_common["ident"] = np.ascontiguousarray(np.eye(128, dtype=np.float32))
    in_maps = []
    for b in range(B):
        m = dict(per_core_common)
        m["x"] = np.ascontiguousarray(x[b])
        m["lat"] = np.ascontiguousarray(lat[b])
        m["maskoff"] = np.ascontiguousarray(maskoff[b])
        in_maps.append(m)
    return in_maps, (gate, same_gamma, has_bm, has_bl)


def _run(inputs, trace=False):
    from concourse.bass_utils import run_bass_kernel_spmd

    in_maps, key = _prep(inputs)
    nc = _cache.get(key)
    if nc is None:
        nc = _build(*key)
        _cache[key] = nc
    res = run_bass_kernel_spmd(nc, in_maps, core_ids=list(range(NCORES)),
                               trace=trace)
    out = np.stack([r["out"] for r in res.results], axis=0).astype(np.float32)
    return out, res


def kernel(**inputs) -> np.ndarray:
    out, _ = _run(inputs, trace=False)
    return out
